# revision 20
# baseline (speedup 1.0000x reference)
"""Trainium2 Bass kernel for batch-8 multi-head attention (B=8, N=1024, C=768, H=12).

Distribution: pure data parallelism — batch element i runs entirely on core i
(weights replicated, zero collectives, full inputs sharded on host).

Design notes (PE-stream-bound; bf16 matmuls, fp32 PSUM):
  - The PE clock ramps: full speed only after ~3us of CONTINUOUS execution,
    and any idle gap resets it to ~half clock. The whole schedule is built
    around one dense, gap-free PE stream: every producer (DMA, convert,
    evac, exp) runs far enough ahead that the PE never waits.
  - W_qkv's q/k blocks are DMA'd as column stripes [768,128] in
    compute-priority order (q0,k0 first) on the sync queue; x tiles first.
    Stage rings are sized so each re-used slot's previous reader is emitted
    before the re-using DMA (the tile framework requires this).
  - x is PE-transposed directly in fp32 (2 cycles/row); the PSUM evacuation
    copy does the fp32->bf16 convert (split DVE/ACT). No separate x cast.
  - q is stored PACKED [128, 6*N] (head pair per 128 partitions); only kT is
    zero-padded per head — the stationary's zero rows kill the moving
    operand's other-head rows, so scores stream full-width 128-contraction.
    Odd heads live in partitions 64:128 of their kT block to stay aligned
    with the packed q.
  - scores^T (s,t) -> exp on ACT (the only engine with Exp; ~117us total,
    the secondary pacer) -> AV with v65 = [v_h | ones] so the softmax
    denominator lands in PSUM column 64 -> DVE reciprocal + per-partition
    tensor_scalar normalize -> PE transpose back -> projection + bias.
  - Software pipeline: scores(h), AV(h-1) and one yT transpose interleave
    per tile index; qk for pair h/2+1 is spread as 512-col half-column
    blocks across the even-head iterations (short PSUM tenancy); stripe
    converts and W_proj DMAs run two heads ahead of their consumers.
  - GpSimd never touches PSUM (unsupported); it does SBUF memsets and
    stays off DVE's shared SBUF write port during the convert-heavy phase.
"""
import numpy as np

import concourse.bacc as bacc
import concourse.bass as bass
import concourse.tile as tile
import concourse.mybir as mybir
from concourse import masks
from concourse.bass_utils import run_bass_kernel_spmd

F32 = mybir.dt.float32
BF16 = mybir.dt.bfloat16

B, N, C = 8, 1024, 768
H, D = 12, 64
SCALE = float(D) ** -0.5
N_CORES = 8
KT = C // 128            # 6 contraction chunks of 128
TT = N // 128            # 8 token tiles of 128
ST = N // 128            # 8 key tiles of 128
PAIRS = H // 2           # 6 head pairs
EXP_FN = mybir.ActivationFunctionType.Exp


def halves(width):
    out = []
    off = 0
    while off < width:
        w = min(512, width - off)
        out.append((off, w))
        off += w
    return out


def build_nc():
    nc = bacc.Bacc("TRN2", target_bir_lowering=False, debug=False,
                   num_devices=N_CORES)
    x_ext = nc.dram_tensor("x", [N, C], F32, kind="ExternalInput")
    wqkv_ext = nc.dram_tensor("W_qkv", [C, 3 * C], F32, kind="ExternalInput")
    wproj_ext = nc.dram_tensor("W_proj", [C, C], F32, kind="ExternalInput")
    bproj_ext = nc.dram_tensor("b_proj", [C], F32, kind="ExternalInput")
    out_ext = nc.dram_tensor("out", [N, C], F32, kind="ExternalOutput")

    with tile.TileContext(nc) as tc:
        with (
            tc.tile_pool(name="const", bufs=1) as constp,
            tc.tile_pool(name="w", bufs=1) as wp_pool,
            tc.tile_pool(name="xt", bufs=1) as xtp,
            tc.tile_pool(name="qk", bufs=1) as qkp,
            tc.tile_pool(name="vp", bufs=1) as vp,
            tc.tile_pool(name="yn", bufs=1) as ynp,
            tc.tile_pool(name="yt", bufs=1) as ytp,
            tc.tile_pool(name="recip", bufs=1) as recipp,
            tc.tile_pool(name="xstage", bufs=1) as xstage,
            tc.tile_pool(name="wstage", bufs=1) as wstage,
            tc.tile_pool(name="exp", bufs=16) as expp,
            tc.tile_pool(name="z", bufs=2) as zp,
            tc.tile_pool(name="psum", bufs=3, space="PSUM") as psum,
        ):
            # ---- constants ----
            ident = constp.tile([128, 128], BF16)
            masks.make_identity(nc, ident[:])
            ident_f = constp.tile([128, 128], F32)
            masks.make_identity(nc, ident_f[:])
            ones_f = constp.tile([1, 128], F32)
            nc.gpsimd.memset(ones_f[:], 1.0)
            b_sb = constp.tile([1, C], F32)
            b_bcast = constp.tile([128, C], BF16)

            # ---- persistent tensors ----
            xt_bf = xtp.tile([128, KT * N], BF16)      # xT: chunk k at [k*N ...]
            # q/k column stripes: stripe i (0-5 = q pair i, 6-11 = k pair i-6)
            # at cols [i*C ...]; within a stripe, chunk k at [k*128 ...]
            wqk_bf = wp_pool.tile([128, 12 * C], BF16)
            wv_bf = wp_pool.tile([128, KT * C], BF16)  # W_v chunk k at [k*C ...]
            wp_bf = wp_pool.tile([128, KT * C], BF16)  # W_proj chunk k at [k*C ...]
            qp_bf = qkp.tile([128, PAIRS * N], BF16)   # packed q: pair p at [p*N ...]
            # kT: head h at [h*N ...]; real rows = (h%2)*64..(h%2)*64+64,
            # the other 64 rows are zeros (stationary pad kills the moving
            # operand's other-head rows in the packed q)
            kt_pad = qkp.tile([128, H * N], BF16)
            v65 = vp.tile([128, ST * H * 65], BF16)    # (s,h) block at [(s*H+h)*65]
            y_nat = ynp.tile([128, TT * C], BF16)      # t-tile t at [t*C], head h at +h*64
            yt_bf = ytp.tile([128, KT * N], BF16)      # yT: chunk c at [c*N ...]

            # ---- staged input DMAs (sync queue, priority order) ----
            # Ring-reuse rule: a DMA into ring slot i may only be EMITTED
            # after the convert reading the slot's previous occupant has been
            # emitted, so DMA emissions are interleaved with the converts at
            # a bounded lookahead instead of all up-front.
            x_tiles = []
            q_stages, k_stages, v_stages, wp_stages = {}, {}, {}, {}

            def stripe_dma(p, kind):
                """DMA a W_qkv column stripe [768,128] into a [128, 768]
                stage (chunk k at cols [k*128 ...])."""
                col0 = p * 128 + (C if kind == "k" else 0)
                st = wstage.tile([128, C], F32, tag="wf", bufs=6)
                src = wqkv_ext[0:C, col0:col0 + 128].rearrange(
                    "(k p) c -> p k c", k=KT)
                dst = st[:].rearrange("p (k c) -> p k c", k=KT)
                nc.sync.dma_start(dst, src)
                (q_stages if kind == "q" else k_stages)[p] = st

            def vchunk_dma(k):
                st = wstage.tile([128, C], F32, tag="wf", bufs=6)
                nc.sync.dma_start(st[:], wqkv_ext[k * 128:(k + 1) * 128,
                                                  2 * C:3 * C])
                v_stages[k] = st

            def wpchunk_dma(k):
                st = wstage.tile([128, C], F32, tag="wf", bufs=6)
                nc.sync.dma_start(st[:], wproj_ext[k * 128:(k + 1) * 128, :])
                wp_stages[k] = st

            def x_dma(t):
                x_f = xstage.tile([128, C], F32, tag="xf", bufs=8)
                nc.sync.dma_start(x_f[:], x_ext[t * 128:(t + 1) * 128, :])
                x_tiles.append(x_f)

            def stripe_cv(p):
                nc.vector.tensor_copy(wqk_bf[:, p * C:(p + 1) * C],
                                      q_stages.pop(p)[:])
                nc.vector.tensor_copy(wqk_bf[:, (6 + p) * C:(7 + p) * C],
                                      k_stages.pop(p)[:])

            with nc.named_scope("dma_in"):
                nc.scalar.dma_start(
                    b_sb[:], bproj_ext[:].rearrange("(a c) -> a c", a=1))
                stripe_dma(0, "q")
                stripe_dma(0, "k")
                for t in range(TT):
                    x_dma(t)
                stripe_dma(1, "q")
                stripe_dma(1, "k")
                vchunk_dma(0)
                vchunk_dma(1)
            stripe_cv(0)

            v65_v = v65[:].rearrange("p (s h w) -> p s h w", h=H, w=65)

            # ---- x: fp32 PE transpose straight from the DMA'd tiles;
            # the evacuation copy does the fp32->bf16 convert (split DVE/ACT)
            def emit_xp(t):
                for k in range(KT):
                    tp = psum.tile([128, 128], F32, tag="ps")
                    nc.tensor.transpose(
                        tp[:], x_tiles[t][:, k * 128:(k + 1) * 128], ident_f[:])
                    if k % 2 == 0:
                        nc.vector.tensor_copy(
                            xt_bf[:, k * N + t * 128: k * N + (t + 1) * 128],
                            tp[:])
                    else:
                        nc.scalar.copy(
                            xt_bf[:, k * N + t * 128: k * N + (t + 1) * 128],
                            tp[:])

            with nc.named_scope("xp"):
                for t in range(4):
                    emit_xp(t)
                vchunk_dma(2)
                vchunk_dma(3)
                for t in range(4, TT):
                    emit_xp(t)
                stripe_cv(1)
                vchunk_dma(4)
                vchunk_dma(5)
                # wv converts early on DVE (v tiles interleave with scores h0)
                for k in range(KT):
                    nc.vector.tensor_copy(wv_bf[:, k * C:(k + 1) * C],
                                          v_stages.pop(k)[:])
                for p in (2, 3, 4):
                    stripe_dma(p, "q")
                    stripe_dma(p, "k")

            # ---- qkv + attention emission helpers ----
            def emit_qk_pair(p):
                # q: one accumulation over chunks, full-width evac (packed)
                q_ps = psum.tile([128, N], F32, tag="ps")
                for k in range(KT):
                    lhsT = wqk_bf[:, p * C + k * 128: p * C + (k + 1) * 128]
                    for off, w in halves(N):
                        nc.tensor.matmul(q_ps[:, off:off + w], lhsT,
                                         xt_bf[:, k * N + off: k * N + off + w],
                                         start=(k == 0), stop=(k == KT - 1))
                nc.vector.tensor_copy(qp_bf[:, p * N:(p + 1) * N], q_ps[:])
                # k: partition-aligned evac into the padded per-head blocks
                k_ps = psum.tile([128, N], F32, tag="ps")
                for k in range(KT):
                    lhsT = wqk_bf[:, (6 + p) * C + k * 128:
                                  (6 + p) * C + (k + 1) * 128]
                    for off, w in halves(N):
                        nc.tensor.matmul(k_ps[:, off:off + w], lhsT,
                                         xt_bf[:, k * N + off: k * N + off + w],
                                         start=(k == 0), stop=(k == KT - 1))
                h0, h1 = 2 * p, 2 * p + 1
                nc.gpsimd.memset(kt_pad[64:128, h0 * N:(h0 + 1) * N], 0.0)
                nc.gpsimd.memset(kt_pad[0:64, h1 * N:(h1 + 1) * N], 0.0)
                nc.vector.tensor_copy(kt_pad[0:64, h0 * N:(h0 + 1) * N],
                                      k_ps[0:64, :])
                nc.vector.tensor_copy(kt_pad[64:128, h1 * N:(h1 + 1) * N],
                                      k_ps[64:128, :])

            def emit_qk_half(p, which, hf):
                off = hf * 512
                base = (p if which == "q" else 6 + p) * C
                ps = psum.tile([128, 512], F32, tag="ps")
                for k in range(KT):
                    lhsT = wqk_bf[:, base + k * 128: base + (k + 1) * 128]
                    nc.tensor.matmul(ps[:, 0:512], lhsT,
                                     xt_bf[:, k * N + off: k * N + off + 512],
                                     start=(k == 0), stop=(k == KT - 1))
                if which == "q":
                    nc.vector.tensor_copy(qp_bf[:, p * N + off: p * N + off + 512],
                                          ps[:])
                else:
                    h0, h1 = 2 * p, 2 * p + 1
                    if hf == 0:
                        nc.gpsimd.memset(kt_pad[64:128, h0 * N:(h0 + 1) * N], 0.0)
                        nc.gpsimd.memset(kt_pad[0:64, h1 * N:(h1 + 1) * N], 0.0)
                    nc.vector.tensor_copy(
                        kt_pad[0:64, h0 * N + off: h0 * N + off + 512],
                        ps[0:64, :])
                    nc.vector.tensor_copy(
                        kt_pad[64:128, h1 * N + off: h1 * N + off + 512],
                        ps[64:128, :])

            e_tiles = {}

            def emit_scores_s(h, s):
                p = h // 2
                s_ps = psum.tile([128, N], F32, tag="ps")
                for off, w in halves(N):
                    nc.tensor.matmul(
                        s_ps[:, off:off + w],
                        kt_pad[:, h * N + s * 128: h * N + (s + 1) * 128],
                        qp_bf[:, p * N + off: p * N + off + w],
                        start=True, stop=True)
                e_t = expp.tile([128, N], BF16, tag="exp")
                nc.scalar.activation(e_t[:], s_ps[:], EXP_FN,
                                     bias=0.0, scale=SCALE)
                e_tiles.setdefault(h, []).append(e_t)


            def emit_v_tile(t):
                v_ps = psum.tile([128, C], F32, tag="ps")
                for k in range(KT):
                    lhsT = xt_bf[:, k * N + t * 128: k * N + (t + 1) * 128]
                    for off, w in halves(C):
                        nc.tensor.matmul(v_ps[:, off:off + w], lhsT,
                                         wv_bf[:, k * C + off: k * C + off + w],
                                         start=(k == 0), stop=(k == KT - 1))
                base = t * H * 65
                v_view = v65[:, base: base + H * 65].rearrange(
                    "p (h w) -> p h w", w=65)
                nc.vector.tensor_copy(
                    v_view[:, :, 0:64],
                    v_ps[:].rearrange("p (h d) -> p h d", d=64))
                nc.gpsimd.memset(v65_v[:, t, :, 64:65], 1.0)

            def emit_av_t(h, t):
                tiles = e_tiles[h]
                y_ps = psum.tile([128, 65], F32, tag="ys", bufs=2)
                for s in range(ST):
                    lhsT = tiles[s][:, t * 128:(t + 1) * 128]
                    rhs = v65[:, (s * H + h) * 65: (s * H + h + 1) * 65]
                    nc.tensor.matmul(y_ps[:, 0:65], lhsT, rhs,
                                     start=(s == 0), stop=(s == ST - 1))
                recip = recipp.tile([128, 1], F32, tag="recip", bufs=4)
                nc.vector.reciprocal(recip[:, 0:1], y_ps[:, 64:65])
                dst = y_nat[:, t * C + h * 64: t * C + (h + 1) * 64]
                nc.vector.tensor_scalar_mul(dst, y_ps[:, 0:64],
                                            recip[:, 0:1])
                if t == TT - 1:
                    e_tiles.pop(h)

            def emit_ytrans_t(i, t):
                tp = psum.tile([128, 128], BF16, tag="ps")
                nc.tensor.transpose(
                    tp[:], y_nat[:, t * C + i * 128: t * C + (i + 1) * 128],
                    ident[:])
                nc.vector.tensor_copy(
                    yt_bf[:, i * N + t * 128: i * N + (t + 1) * 128], tp[:])

            # ---- phase: qkv head-pair 0 + b broadcast ----
            _qs = nc.enter_named_scope("qkv", False)
            # b broadcast to 128 partitions via fp32 PE matmul (tiny, also
            # warms the PE pstate ramp before the dense qk stream)
            b_ps = psum.tile([128, C], F32, tag="ps")
            for off, w in halves(C):
                nc.tensor.matmul(b_ps[:, off:off + w], ones_f[0:1, 0:128],
                                 b_sb[0:1, off:off + w], start=True, stop=True)
            nc.scalar.copy(b_bcast[:], b_ps[:])
            emit_qk_pair(0)

            emit_qk_pair(1)
            stripe_cv(2)
            stripe_dma(5, "q")
            stripe_dma(5, "k")
            for i in range(TT):
                emit_scores_s(0, i)
                emit_v_tile(i)
            for i in range(TT):
                emit_scores_s(1, i)
                if i >= 1:
                    emit_av_t(0, i - 1)
            nc.leave_named_scope("qkv", _qs[0], False)

            # ---- phase: attention, software-pipelined across heads ----
            # scores(h) and AV(h-1) interleave per tile index so the AV
            # results arrive spread out (less DVE head-blocking) and the PE
            # stream stays dense (pstate ramp: any PE gap resets the clock
            # to mid for ~3us)
            _as = nc.enter_named_scope("attn", False)
            emit_av_t(0, TT - 1)
            spread_pairs = {2: 2, 4: 3, 6: 4, 9: 5}
            for h in range(2, H):
                spread_pair = spread_pairs.get(h)
                for i in range(TT):
                    emit_scores_s(h, i)
                    emit_av_t(h - 1, i)
                    if h in (3, 5, 7):
                        emit_ytrans_t((h - 3) // 2, i)
                    elif h == 10:
                        # yT3 here, yT4 one step skewed (norm h9 lands during
                        # this iteration); both fill the otherwise-bare head
                        emit_ytrans_t(3, i)
                        if i >= 1:
                            emit_ytrans_t(4, i - 1)
                    elif h == 11 and i == 0:
                        emit_ytrans_t(4, TT - 1)
                    if spread_pair is not None:
                        if h == 9:
                            if i % 2 == 0:
                                which, hf = (("q", 0), ("q", 1), ("k", 0),
                                             ("k", 1))[i // 2]
                                emit_qk_half(spread_pair, which, hf)
                        elif i % 2 == 1:
                            which, hf = (("q", 0), ("q", 1), ("k", 0),
                                         ("k", 1))[i // 2]
                            emit_qk_half(spread_pair, which, hf)
                if h in (2, 4, 6):
                    # stripe converts + wp DMAs run two heads ahead of the
                    # pair matmuls so the PE never waits on them
                    stripe_cv(h // 2 + 2)
                    if h < 6:
                        wpchunk_dma(h - 2)
                        wpchunk_dma(h - 1)
                if h == 8:
                    wpchunk_dma(4)
                    wpchunk_dma(5)
                if h in (9, 10, 11):
                    k0 = 2 * (h - 9)
                    nc.vector.tensor_copy(wp_bf[:, k0 * C:(k0 + 1) * C],
                                          wp_stages.pop(k0)[:])
                    nc.vector.tensor_copy(wp_bf[:, (k0 + 1) * C:(k0 + 2) * C],
                                          wp_stages.pop(k0 + 1)[:])
            for i in range(TT):
                emit_av_t(H - 1, i)
                if i >= 1:
                    emit_ytrans_t(5, i - 1)
            emit_ytrans_t(5, TT - 1)
            nc.leave_named_scope("attn", _as[0], False)

            # ---- phase: out = yT^T @ W_proj + b ----
            _ps_ = nc.enter_named_scope("proj", False)
            for t in range(TT):
                z_ps = psum.tile([128, C], F32, tag="ps")
                for k in range(KT):
                    lhsT = yt_bf[:, k * N + t * 128: k * N + (t + 1) * 128]
                    for off, w in halves(C):
                        nc.tensor.matmul(z_ps[:, off:off + w], lhsT,
                                         wp_bf[:, k * C + off: k * C + off + w],
                                         start=(k == 0), stop=(k == KT - 1))
                z_sb = zp.tile([128, C], F32, tag="z")
                half = C // 2
                for hf in range(2):
                    cs = slice(hf * half, (hf + 1) * half)
                    nc.vector.tensor_add(z_sb[:, cs], z_ps[:, cs],
                                         b_bcast[:, cs])
                    nc.scalar.dma_start(out_ext[t * 128:(t + 1) * 128, cs],
                                        z_sb[:, cs])
            nc.leave_named_scope("proj", _ps_[0], False)

    nc.finalize()
    return nc


_NC = None


def _get_nc():
    global _NC
    if _NC is None:
        _NC = build_nc()
    return _NC


def _run(x, W_qkv, W_proj, b_proj, trace=False):
    nc = _get_nc()
    W_qkv = np.ascontiguousarray(W_qkv, dtype=np.float32)
    W_proj = np.ascontiguousarray(W_proj, dtype=np.float32)
    b_proj = np.ascontiguousarray(b_proj, dtype=np.float32)
    in_maps = [
        {
            "x": np.ascontiguousarray(x[i], dtype=np.float32),
            "W_qkv": W_qkv,
            "W_proj": W_proj,
            "b_proj": b_proj,
        }
        for i in range(N_CORES)
    ]
    res = run_bass_kernel_spmd(nc, in_maps, core_ids=list(range(N_CORES)),
                               trace=trace)
    out = np.stack([res.results[i]["out"] for i in range(N_CORES)], axis=0)
    return out.astype(np.float32), res


def kernel(x, W_qkv, W_proj, b_proj):
    out, _ = _run(x, W_qkv, W_proj, b_proj, trace=False)
    return out


# revision 21
# speedup vs baseline: 1.0152x; 1.0152x over previous
"""Trainium2 Bass kernel for batch-8 multi-head attention (B=8, N=1024, C=768, H=12).

Distribution: pure data parallelism — batch element i runs entirely on core i
(weights replicated, zero collectives, full inputs sharded on host).

Design notes (PE-stream-bound; bf16 matmuls, fp32 PSUM):
  - The PE clock ramps: full speed only after ~3us of CONTINUOUS execution,
    and any idle gap resets it to ~half clock. The whole schedule is built
    around one dense, gap-free PE stream: every producer (DMA, convert,
    evac, exp) runs far enough ahead that the PE never waits.
  - W_qkv's q/k blocks are DMA'd as column stripes [768,128] in
    compute-priority order (q0,k0 first) on the sync queue; x tiles first.
    Stage rings are sized so each re-used slot's previous reader is emitted
    before the re-using DMA (the tile framework requires this).
  - x is PE-transposed directly in fp32 (2 cycles/row); the PSUM evacuation
    copy does the fp32->bf16 convert (split DVE/ACT). No separate x cast.
  - q is stored PACKED [128, 6*N] (head pair per 128 partitions); only kT is
    zero-padded per head — the stationary's zero rows kill the moving
    operand's other-head rows, so scores stream full-width 128-contraction.
    Odd heads live in partitions 64:128 of their kT block to stay aligned
    with the packed q.
  - scores^T (s,t) -> exp on ACT (the only engine with Exp; ~117us total,
    the secondary pacer) -> AV with v65 = [v_h | ones] so the softmax
    denominator lands in PSUM column 64 -> DVE reciprocal + per-partition
    tensor_scalar normalize -> PE transpose back -> projection + bias.
  - Software pipeline: scores(h), AV(h-1) and one yT transpose interleave
    per tile index; qk for pair h/2+1 is spread as 512-col half-column
    blocks across the even-head iterations (short PSUM tenancy); stripe
    converts and W_proj DMAs run two heads ahead of their consumers.
  - GpSimd never touches PSUM (unsupported); it does SBUF memsets and
    stays off DVE's shared SBUF write port during the convert-heavy phase.
"""
import numpy as np

import concourse.bacc as bacc
import concourse.bass as bass
import concourse.tile as tile
import concourse.mybir as mybir
from concourse import masks
from concourse.bass_utils import run_bass_kernel_spmd

F32 = mybir.dt.float32
BF16 = mybir.dt.bfloat16

B, N, C = 8, 1024, 768
H, D = 12, 64
SCALE = float(D) ** -0.5
N_CORES = 8
KT = C // 128            # 6 contraction chunks of 128
TT = N // 128            # 8 token tiles of 128
ST = N // 128            # 8 key tiles of 128
PAIRS = H // 2           # 6 head pairs
EXP_FN = mybir.ActivationFunctionType.Exp


def halves(width):
    out = []
    off = 0
    while off < width:
        w = min(512, width - off)
        out.append((off, w))
        off += w
    return out


def build_nc():
    nc = bacc.Bacc("TRN2", target_bir_lowering=False, debug=False,
                   num_devices=N_CORES)
    x_ext = nc.dram_tensor("x", [N, C], F32, kind="ExternalInput")
    wqkv_ext = nc.dram_tensor("W_qkv", [C, 3 * C], F32, kind="ExternalInput")
    wproj_ext = nc.dram_tensor("W_proj", [C, C], F32, kind="ExternalInput")
    bproj_ext = nc.dram_tensor("b_proj", [C], F32, kind="ExternalInput")
    out_ext = nc.dram_tensor("out", [N, C], F32, kind="ExternalOutput")

    with tile.TileContext(nc) as tc:
        with (
            tc.tile_pool(name="const", bufs=1) as constp,
            tc.tile_pool(name="w", bufs=1) as wp_pool,
            tc.tile_pool(name="xt", bufs=1) as xtp,
            tc.tile_pool(name="qk", bufs=1) as qkp,
            tc.tile_pool(name="vp", bufs=1) as vp,
            tc.tile_pool(name="yn", bufs=1) as ynp,
            tc.tile_pool(name="yt", bufs=1) as ytp,
            tc.tile_pool(name="recip", bufs=1) as recipp,
            tc.tile_pool(name="xstage", bufs=1) as xstage,
            tc.tile_pool(name="wstage", bufs=1) as wstage,
            tc.tile_pool(name="exp", bufs=16) as expp,
            tc.tile_pool(name="z", bufs=2) as zp,
            tc.tile_pool(name="psum", bufs=3, space="PSUM") as psum,
        ):
            # ---- constants ----
            ident = constp.tile([128, 128], BF16)
            masks.make_identity(nc, ident[:])
            ident_f = constp.tile([128, 128], F32)
            masks.make_identity(nc, ident_f[:])
            ones_f = constp.tile([1, 128], F32)
            nc.gpsimd.memset(ones_f[:], 1.0)
            b_sb = constp.tile([1, C], F32)
            b_bcast = constp.tile([128, C], BF16)

            # ---- persistent tensors ----
            xt_bf = xtp.tile([128, KT * N], BF16)      # xT: chunk k at [k*N ...]
            # q/k column stripes: stripe i (0-5 = q pair i, 6-11 = k pair i-6)
            # at cols [i*C ...]; within a stripe, chunk k at [k*128 ...]
            wqk_bf = wp_pool.tile([128, 12 * C], BF16)
            wv_bf = wp_pool.tile([128, KT * C], BF16)  # W_v chunk k at [k*C ...]
            wp_bf = wp_pool.tile([128, KT * C], BF16)  # W_proj chunk k at [k*C ...]
            qp_bf = qkp.tile([128, PAIRS * N], BF16)   # packed q: pair p at [p*N ...]
            # kT: head h at [h*N ...]; real rows = (h%2)*64..(h%2)*64+64,
            # the other 64 rows are zeros (stationary pad kills the moving
            # operand's other-head rows in the packed q)
            kt_pad = qkp.tile([128, H * N], BF16)
            v65 = vp.tile([128, ST * H * 65], BF16)    # (s,h) block at [(s*H+h)*65]
            y_nat = ynp.tile([128, TT * C], BF16)      # t-tile t at [t*C], head h at +h*64
            yt_bf = ytp.tile([128, KT * N], BF16)      # yT: chunk c at [c*N ...]

            # ---- staged input DMAs (sync queue, priority order) ----
            # Ring-reuse rule: a DMA into ring slot i may only be EMITTED
            # after the convert reading the slot's previous occupant has been
            # emitted, so DMA emissions are interleaved with the converts at
            # a bounded lookahead instead of all up-front.
            x_tiles = []
            q_stages, k_stages, v_stages, wp_stages = {}, {}, {}, {}

            def stripe_dma(p, kind):
                """DMA a W_qkv column stripe [768,128] into a [128, 768]
                stage (chunk k at cols [k*128 ...])."""
                col0 = p * 128 + (C if kind == "k" else 0)
                st = wstage.tile([128, C], F32, tag="wf", bufs=6)
                src = wqkv_ext[0:C, col0:col0 + 128].rearrange(
                    "(k p) c -> p k c", k=KT)
                dst = st[:].rearrange("p (k c) -> p k c", k=KT)
                nc.sync.dma_start(dst, src)
                (q_stages if kind == "q" else k_stages)[p] = st

            def vchunk_dma(k):
                st = wstage.tile([128, C], F32, tag="wf", bufs=6)
                nc.sync.dma_start(st[:], wqkv_ext[k * 128:(k + 1) * 128,
                                                  2 * C:3 * C])
                v_stages[k] = st

            def wpchunk_dma(k):
                st = wstage.tile([128, C], F32, tag="wf", bufs=6)
                nc.sync.dma_start(st[:], wproj_ext[k * 128:(k + 1) * 128, :])
                wp_stages[k] = st

            def x_dma(t):
                x_f = xstage.tile([128, C], F32, tag="xf", bufs=8)
                nc.sync.dma_start(x_f[:], x_ext[t * 128:(t + 1) * 128, :])
                x_tiles.append(x_f)

            def stripe_cv(p):
                nc.vector.tensor_copy(wqk_bf[:, p * C:(p + 1) * C],
                                      q_stages.pop(p)[:])
                nc.vector.tensor_copy(wqk_bf[:, (6 + p) * C:(7 + p) * C],
                                      k_stages.pop(p)[:])

            with nc.named_scope("dma_in"):
                nc.scalar.dma_start(
                    b_sb[:], bproj_ext[:].rearrange("(a c) -> a c", a=1))
                for t in range(TT):
                    x_dma(t)
                stripe_dma(0, "q")
                stripe_dma(0, "k")
                stripe_dma(1, "q")
                stripe_dma(1, "k")
                vchunk_dma(0)
                vchunk_dma(1)

            v65_v = v65[:].rearrange("p (s h w) -> p s h w", h=H, w=65)

            # ---- x: fp32 PE transpose straight from the DMA'd tiles;
            # the evacuation copy does the fp32->bf16 convert (split DVE/ACT)
            def emit_xp(t):
                for k in range(KT):
                    tp = psum.tile([128, 128], F32, tag="ps")
                    nc.tensor.transpose(
                        tp[:], x_tiles[t][:, k * 128:(k + 1) * 128], ident_f[:])
                    if k % 2 == 0:
                        nc.vector.tensor_copy(
                            xt_bf[:, k * N + t * 128: k * N + (t + 1) * 128],
                            tp[:])
                    else:
                        nc.scalar.copy(
                            xt_bf[:, k * N + t * 128: k * N + (t + 1) * 128],
                            tp[:])

            with nc.named_scope("xp"):
                for t in range(4):
                    emit_xp(t)
                stripe_cv(0)
                vchunk_dma(2)
                vchunk_dma(3)
                for t in range(4, TT):
                    emit_xp(t)
                stripe_cv(1)
                vchunk_dma(4)
                vchunk_dma(5)
                # wv converts early on DVE (v tiles interleave with scores h0)
                for k in range(KT):
                    nc.vector.tensor_copy(wv_bf[:, k * C:(k + 1) * C],
                                          v_stages.pop(k)[:])
                for p in (2, 3, 4):
                    stripe_dma(p, "q")
                    stripe_dma(p, "k")

            # ---- qkv + attention emission helpers ----
            def emit_qk_pair(p):
                # q: one accumulation over chunks, full-width evac (packed)
                q_ps = psum.tile([128, N], F32, tag="ps")
                for k in range(KT):
                    lhsT = wqk_bf[:, p * C + k * 128: p * C + (k + 1) * 128]
                    for off, w in halves(N):
                        nc.tensor.matmul(q_ps[:, off:off + w], lhsT,
                                         xt_bf[:, k * N + off: k * N + off + w],
                                         start=(k == 0), stop=(k == KT - 1))
                nc.vector.tensor_copy(qp_bf[:, p * N:(p + 1) * N], q_ps[:])
                # k: partition-aligned evac into the padded per-head blocks
                k_ps = psum.tile([128, N], F32, tag="ps")
                for k in range(KT):
                    lhsT = wqk_bf[:, (6 + p) * C + k * 128:
                                  (6 + p) * C + (k + 1) * 128]
                    for off, w in halves(N):
                        nc.tensor.matmul(k_ps[:, off:off + w], lhsT,
                                         xt_bf[:, k * N + off: k * N + off + w],
                                         start=(k == 0), stop=(k == KT - 1))
                h0, h1 = 2 * p, 2 * p + 1
                nc.gpsimd.memset(kt_pad[64:128, h0 * N:(h0 + 1) * N], 0.0)
                nc.gpsimd.memset(kt_pad[0:64, h1 * N:(h1 + 1) * N], 0.0)
                nc.vector.tensor_copy(kt_pad[0:64, h0 * N:(h0 + 1) * N],
                                      k_ps[0:64, :])
                nc.vector.tensor_copy(kt_pad[64:128, h1 * N:(h1 + 1) * N],
                                      k_ps[64:128, :])

            def emit_qk_half(p, which, hf):
                off = hf * 512
                base = (p if which == "q" else 6 + p) * C
                ps = psum.tile([128, 512], F32, tag="ps")
                for k in range(KT):
                    lhsT = wqk_bf[:, base + k * 128: base + (k + 1) * 128]
                    nc.tensor.matmul(ps[:, 0:512], lhsT,
                                     xt_bf[:, k * N + off: k * N + off + 512],
                                     start=(k == 0), stop=(k == KT - 1))
                if which == "q":
                    nc.vector.tensor_copy(qp_bf[:, p * N + off: p * N + off + 512],
                                          ps[:])
                else:
                    h0, h1 = 2 * p, 2 * p + 1
                    if hf == 0:
                        nc.gpsimd.memset(kt_pad[64:128, h0 * N:(h0 + 1) * N], 0.0)
                        nc.gpsimd.memset(kt_pad[0:64, h1 * N:(h1 + 1) * N], 0.0)
                    nc.vector.tensor_copy(
                        kt_pad[0:64, h0 * N + off: h0 * N + off + 512],
                        ps[0:64, :])
                    nc.vector.tensor_copy(
                        kt_pad[64:128, h1 * N + off: h1 * N + off + 512],
                        ps[64:128, :])

            e_tiles = {}

            def emit_scores_s(h, s):
                p = h // 2
                s_ps = psum.tile([128, N], F32, tag="ps")
                for off, w in halves(N):
                    nc.tensor.matmul(
                        s_ps[:, off:off + w],
                        kt_pad[:, h * N + s * 128: h * N + (s + 1) * 128],
                        qp_bf[:, p * N + off: p * N + off + w],
                        start=True, stop=True)
                e_t = expp.tile([128, N], BF16, tag="exp")
                nc.scalar.activation(e_t[:], s_ps[:], EXP_FN,
                                     bias=0.0, scale=SCALE)
                e_tiles.setdefault(h, []).append(e_t)


            def emit_v_tile(t):
                v_ps = psum.tile([128, C], F32, tag="ps")
                for k in range(KT):
                    lhsT = xt_bf[:, k * N + t * 128: k * N + (t + 1) * 128]
                    for off, w in halves(C):
                        nc.tensor.matmul(v_ps[:, off:off + w], lhsT,
                                         wv_bf[:, k * C + off: k * C + off + w],
                                         start=(k == 0), stop=(k == KT - 1))
                base = t * H * 65
                v_view = v65[:, base: base + H * 65].rearrange(
                    "p (h w) -> p h w", w=65)
                nc.vector.tensor_copy(
                    v_view[:, :, 0:64],
                    v_ps[:].rearrange("p (h d) -> p h d", d=64))
                nc.gpsimd.memset(v65_v[:, t, :, 64:65], 1.0)

            def emit_av_t(h, t):
                tiles = e_tiles[h]
                y_ps = psum.tile([128, 65], F32, tag="ys", bufs=2)
                for s in range(ST):
                    lhsT = tiles[s][:, t * 128:(t + 1) * 128]
                    rhs = v65[:, (s * H + h) * 65: (s * H + h + 1) * 65]
                    nc.tensor.matmul(y_ps[:, 0:65], lhsT, rhs,
                                     start=(s == 0), stop=(s == ST - 1))
                recip = recipp.tile([128, 1], F32, tag="recip", bufs=4)
                nc.vector.reciprocal(recip[:, 0:1], y_ps[:, 64:65])
                dst = y_nat[:, t * C + h * 64: t * C + (h + 1) * 64]
                nc.vector.tensor_scalar_mul(dst, y_ps[:, 0:64],
                                            recip[:, 0:1])
                if t == TT - 1:
                    e_tiles.pop(h)

            def emit_ytrans_t(i, t):
                tp = psum.tile([128, 128], BF16, tag="ps")
                nc.tensor.transpose(
                    tp[:], y_nat[:, t * C + i * 128: t * C + (i + 1) * 128],
                    ident[:])
                nc.vector.tensor_copy(
                    yt_bf[:, i * N + t * 128: i * N + (t + 1) * 128], tp[:])

            # ---- phase: qkv head-pair 0 + b broadcast ----
            _qs = nc.enter_named_scope("qkv", False)
            # b broadcast to 128 partitions via fp32 PE matmul (tiny, also
            # warms the PE pstate ramp before the dense qk stream)
            b_ps = psum.tile([128, C], F32, tag="ps")
            for off, w in halves(C):
                nc.tensor.matmul(b_ps[:, off:off + w], ones_f[0:1, 0:128],
                                 b_sb[0:1, off:off + w], start=True, stop=True)
            nc.scalar.copy(b_bcast[:], b_ps[:])
            emit_qk_pair(0)

            emit_qk_pair(1)
            stripe_cv(2)
            stripe_dma(5, "q")
            stripe_dma(5, "k")
            for i in range(TT):
                emit_scores_s(0, i)
                emit_v_tile(i)
            for i in range(TT):
                emit_scores_s(1, i)
                if i >= 1:
                    emit_av_t(0, i - 1)
            nc.leave_named_scope("qkv", _qs[0], False)

            # ---- phase: attention, software-pipelined across heads ----
            # scores(h) and AV(h-1) interleave per tile index so the AV
            # results arrive spread out (less DVE head-blocking) and the PE
            # stream stays dense (pstate ramp: any PE gap resets the clock
            # to mid for ~3us)
            _as = nc.enter_named_scope("attn", False)
            emit_av_t(0, TT - 1)
            spread_pairs = {2: 2, 4: 3, 6: 4, 9: 5}
            for h in range(2, H):
                spread_pair = spread_pairs.get(h)
                for i in range(TT):
                    emit_scores_s(h, i)
                    emit_av_t(h - 1, i)
                    if h in (3, 5, 7):
                        emit_ytrans_t((h - 3) // 2, i)
                    elif h == 10:
                        # yT3 here, yT4 one step skewed (norm h9 lands during
                        # this iteration); both fill the otherwise-bare head
                        emit_ytrans_t(3, i)
                        if i >= 1:
                            emit_ytrans_t(4, i - 1)
                    elif h == 11 and i == 0:
                        emit_ytrans_t(4, TT - 1)
                    if spread_pair is not None:
                        if h == 9:
                            if i % 2 == 0:
                                which, hf = (("q", 0), ("q", 1), ("k", 0),
                                             ("k", 1))[i // 2]
                                emit_qk_half(spread_pair, which, hf)
                        elif i % 2 == 1:
                            which, hf = (("q", 0), ("q", 1), ("k", 0),
                                         ("k", 1))[i // 2]
                            emit_qk_half(spread_pair, which, hf)
                if h in (2, 4, 6):
                    # stripe converts + wp DMAs run two heads ahead of the
                    # pair matmuls so the PE never waits on them
                    stripe_cv(h // 2 + 2)
                    if h < 6:
                        wpchunk_dma(h - 2)
                        wpchunk_dma(h - 1)
                if h == 8:
                    wpchunk_dma(4)
                    wpchunk_dma(5)
                if h in (9, 10, 11):
                    k0 = 2 * (h - 9)
                    nc.vector.tensor_copy(wp_bf[:, k0 * C:(k0 + 1) * C],
                                          wp_stages.pop(k0)[:])
                    nc.vector.tensor_copy(wp_bf[:, (k0 + 1) * C:(k0 + 2) * C],
                                          wp_stages.pop(k0 + 1)[:])
            for i in range(TT):
                emit_av_t(H - 1, i)
                if i >= 1:
                    emit_ytrans_t(5, i - 1)
            emit_ytrans_t(5, TT - 1)
            nc.leave_named_scope("attn", _as[0], False)

            # ---- phase: out = yT^T @ W_proj + b ----
            _ps_ = nc.enter_named_scope("proj", False)
            for t in range(TT):
                z_ps = psum.tile([128, C], F32, tag="ps")
                for k in range(KT):
                    lhsT = yt_bf[:, k * N + t * 128: k * N + (t + 1) * 128]
                    for off, w in halves(C):
                        nc.tensor.matmul(z_ps[:, off:off + w], lhsT,
                                         wp_bf[:, k * C + off: k * C + off + w],
                                         start=(k == 0), stop=(k == KT - 1))
                z_sb = zp.tile([128, C], F32, tag="z")
                half = C // 2
                for hf in range(2):
                    cs = slice(hf * half, (hf + 1) * half)
                    nc.vector.tensor_add(z_sb[:, cs], z_ps[:, cs],
                                         b_bcast[:, cs])
                    nc.scalar.dma_start(out_ext[t * 128:(t + 1) * 128, cs],
                                        z_sb[:, cs])
            nc.leave_named_scope("proj", _ps_[0], False)

    nc.finalize()
    return nc


_NC = None


def _get_nc():
    global _NC
    if _NC is None:
        _NC = build_nc()
    return _NC


def _run(x, W_qkv, W_proj, b_proj, trace=False):
    nc = _get_nc()
    W_qkv = np.ascontiguousarray(W_qkv, dtype=np.float32)
    W_proj = np.ascontiguousarray(W_proj, dtype=np.float32)
    b_proj = np.ascontiguousarray(b_proj, dtype=np.float32)
    in_maps = [
        {
            "x": np.ascontiguousarray(x[i], dtype=np.float32),
            "W_qkv": W_qkv,
            "W_proj": W_proj,
            "b_proj": b_proj,
        }
        for i in range(N_CORES)
    ]
    res = run_bass_kernel_spmd(nc, in_maps, core_ids=list(range(N_CORES)),
                               trace=trace)
    out = np.stack([res.results[i]["out"] for i in range(N_CORES)], axis=0)
    return out.astype(np.float32), res


def kernel(x, W_qkv, W_proj, b_proj):
    out, _ = _run(x, W_qkv, W_proj, b_proj, trace=False)
    return out


# revision 22
# speedup vs baseline: 1.0237x; 1.0084x over previous
"""Trainium2 Bass kernel for batch-8 multi-head attention (B=8, N=1024, C=768, H=12).

Distribution: pure data parallelism — batch element i runs entirely on core i
(weights replicated, zero collectives, full inputs sharded on host).

Design notes (PE-stream-bound; bf16 matmuls, fp32 PSUM):
  - The PE clock ramps: full speed only after ~3us of CONTINUOUS execution,
    and any idle gap resets it to ~half clock. The whole schedule is built
    around one dense, gap-free PE stream: every producer (DMA, convert,
    evac, exp) runs far enough ahead that the PE never waits.
  - W_qkv's q/k blocks are DMA'd as column stripes [768,128] in
    compute-priority order (q0,k0 first) on the sync queue; x tiles first.
    Stage rings are sized so each re-used slot's previous reader is emitted
    before the re-using DMA (the tile framework requires this).
  - x is PE-transposed directly in fp32 (2 cycles/row); the PSUM evacuation
    copy does the fp32->bf16 convert (split DVE/ACT). No separate x cast.
  - q is stored PACKED [128, 6*N] (head pair per 128 partitions); only kT is
    zero-padded per head — the stationary's zero rows kill the moving
    operand's other-head rows, so scores stream full-width 128-contraction.
    Odd heads live in partitions 64:128 of their kT block to stay aligned
    with the packed q.
  - scores^T (s,t) -> exp on ACT (the only engine with Exp; ~117us total,
    the secondary pacer) -> AV with v65 = [v_h | ones] so the softmax
    denominator lands in PSUM column 64 -> DVE reciprocal + per-partition
    tensor_scalar normalize -> PE transpose back -> projection + bias.
  - Software pipeline: scores(h), AV(h-1) and one yT transpose interleave
    per tile index; qk for pair h/2+1 is spread as 512-col half-column
    blocks across the even-head iterations (short PSUM tenancy); stripe
    converts and W_proj DMAs run two heads ahead of their consumers.
  - GpSimd never touches PSUM (unsupported); it does SBUF memsets and
    stays off DVE's shared SBUF write port during the convert-heavy phase.
"""
import numpy as np

import concourse.bacc as bacc
import concourse.bass as bass
import concourse.tile as tile
import concourse.mybir as mybir
from concourse import masks
from concourse.bass_utils import run_bass_kernel_spmd

F32 = mybir.dt.float32
BF16 = mybir.dt.bfloat16

B, N, C = 8, 1024, 768
H, D = 12, 64
SCALE = float(D) ** -0.5
N_CORES = 8
KT = C // 128            # 6 contraction chunks of 128
TT = N // 128            # 8 token tiles of 128
ST = N // 128            # 8 key tiles of 128
PAIRS = H // 2           # 6 head pairs
EXP_FN = mybir.ActivationFunctionType.Exp


def halves(width):
    out = []
    off = 0
    while off < width:
        w = min(512, width - off)
        out.append((off, w))
        off += w
    return out


def build_nc():
    nc = bacc.Bacc("TRN2", target_bir_lowering=False, debug=False,
                   num_devices=N_CORES)
    x_ext = nc.dram_tensor("x", [N, C], F32, kind="ExternalInput")
    wqkv_ext = nc.dram_tensor("W_qkv", [C, 3 * C], F32, kind="ExternalInput")
    wproj_ext = nc.dram_tensor("W_proj", [C, C], F32, kind="ExternalInput")
    bproj_ext = nc.dram_tensor("b_proj", [C], F32, kind="ExternalInput")
    out_ext = nc.dram_tensor("out", [N, C], F32, kind="ExternalOutput")

    with tile.TileContext(nc) as tc:
        with (
            tc.tile_pool(name="const", bufs=1) as constp,
            tc.tile_pool(name="w", bufs=1) as wp_pool,
            tc.tile_pool(name="xt", bufs=1) as xtp,
            tc.tile_pool(name="qk", bufs=1) as qkp,
            tc.tile_pool(name="vp", bufs=1) as vp,
            tc.tile_pool(name="yn", bufs=1) as ynp,
            tc.tile_pool(name="yt", bufs=1) as ytp,
            tc.tile_pool(name="recip", bufs=1) as recipp,
            tc.tile_pool(name="xstage", bufs=1) as xstage,
            tc.tile_pool(name="wstage", bufs=1) as wstage,
            tc.tile_pool(name="exp", bufs=16) as expp,
            tc.tile_pool(name="z", bufs=2) as zp,
            tc.tile_pool(name="psum", bufs=3, space="PSUM") as psum,
        ):
            # ---- constants ----
            ident = constp.tile([128, 128], BF16)
            masks.make_identity(nc, ident[:])
            ident_f = constp.tile([128, 128], F32)
            masks.make_identity(nc, ident_f[:])
            ones_f = constp.tile([1, 128], F32)
            nc.gpsimd.memset(ones_f[:], 1.0)
            b_sb = constp.tile([1, C], F32)
            b_bcast = constp.tile([128, C], BF16)

            # ---- persistent tensors ----
            xt_bf = xtp.tile([128, KT * N], BF16)      # xT: chunk k at [k*N ...]
            # q/k column stripes: stripe i (0-5 = q pair i, 6-11 = k pair i-6)
            # at cols [i*C ...]; within a stripe, chunk k at [k*128 ...]
            wqk_bf = wp_pool.tile([128, 12 * C], BF16)
            wv_bf = wp_pool.tile([128, KT * C], BF16)  # W_v chunk k at [k*C ...]
            wp_bf = wp_pool.tile([128, KT * C], BF16)  # W_proj chunk k at [k*C ...]
            qp_bf = qkp.tile([128, PAIRS * N], BF16)   # packed q: pair p at [p*N ...]
            # kT: head h at [h*N ...]; real rows = (h%2)*64..(h%2)*64+64,
            # the other 64 rows are zeros (stationary pad kills the moving
            # operand's other-head rows in the packed q)
            kt_pad = qkp.tile([128, H * N], BF16)
            v65 = vp.tile([128, ST * H * 65], BF16)    # (s,h) block at [(s*H+h)*65]
            y_nat = ynp.tile([128, TT * C], BF16)      # t-tile t at [t*C], head h at +h*64
            yt_bf = ytp.tile([128, KT * N], BF16)      # yT: chunk c at [c*N ...]

            # ---- staged input DMAs (sync queue, priority order) ----
            # Ring-reuse rule: a DMA into ring slot i may only be EMITTED
            # after the convert reading the slot's previous occupant has been
            # emitted, so DMA emissions are interleaved with the converts at
            # a bounded lookahead instead of all up-front.
            x_tiles = []
            q_stages, k_stages, v_stages, wp_stages = {}, {}, {}, {}

            def stripe_dma(p, kind):
                """DMA a W_qkv column stripe [768,128] into a [128, 768]
                stage (chunk k at cols [k*128 ...])."""
                col0 = p * 128 + (C if kind == "k" else 0)
                st = wstage.tile([128, C], F32, tag="wf", bufs=6)
                src = wqkv_ext[0:C, col0:col0 + 128].rearrange(
                    "(k p) c -> p k c", k=KT)
                dst = st[:].rearrange("p (k c) -> p k c", k=KT)
                nc.sync.dma_start(dst, src)
                (q_stages if kind == "q" else k_stages)[p] = st

            def vchunk_dma(k):
                st = wstage.tile([128, C], F32, tag="wf", bufs=6)
                nc.sync.dma_start(st[:], wqkv_ext[k * 128:(k + 1) * 128,
                                                  2 * C:3 * C])
                v_stages[k] = st

            def wpchunk_dma(k):
                st = wstage.tile([128, C], F32, tag="wf", bufs=6)
                nc.sync.dma_start(st[:], wproj_ext[k * 128:(k + 1) * 128, :])
                wp_stages[k] = st

            def x_dma(t):
                x_f = xstage.tile([128, C], F32, tag="xf", bufs=8)
                nc.sync.dma_start(x_f[:], x_ext[t * 128:(t + 1) * 128, :])
                x_tiles.append(x_f)

            def stripe_cv(p):
                nc.vector.tensor_copy(wqk_bf[:, p * C:(p + 1) * C],
                                      q_stages.pop(p)[:])
                nc.vector.tensor_copy(wqk_bf[:, (6 + p) * C:(7 + p) * C],
                                      k_stages.pop(p)[:])

            with nc.named_scope("dma_in"):
                nc.scalar.dma_start(
                    b_sb[:], bproj_ext[:].rearrange("(a c) -> a c", a=1))
                for t in range(TT):
                    x_dma(t)
                stripe_dma(0, "q")
                stripe_dma(0, "k")
                stripe_dma(1, "q")
                stripe_dma(1, "k")
                vchunk_dma(0)
                vchunk_dma(1)

            v65_v = v65[:].rearrange("p (s h w) -> p s h w", h=H, w=65)

            # ---- x: fp32 PE transpose straight from the DMA'd tiles;
            # the evacuation copy does the fp32->bf16 convert (split DVE/ACT)
            def emit_xp(t):
                for k in range(KT):
                    tp = psum.tile([128, 128], F32, tag="ps")
                    nc.tensor.transpose(
                        tp[:], x_tiles[t][:, k * 128:(k + 1) * 128], ident_f[:])
                    if k % 2 == 0:
                        nc.vector.tensor_copy(
                            xt_bf[:, k * N + t * 128: k * N + (t + 1) * 128],
                            tp[:])
                    else:
                        nc.scalar.copy(
                            xt_bf[:, k * N + t * 128: k * N + (t + 1) * 128],
                            tp[:])

            with nc.named_scope("xp"):
                for t in range(4):
                    emit_xp(t)
                stripe_cv(0)
                vchunk_dma(2)
                vchunk_dma(3)
                for t in range(4, TT):
                    emit_xp(t)
                stripe_cv(1)
                vchunk_dma(4)
                vchunk_dma(5)
                # wv converts early on DVE (v tiles interleave with scores h0)
                for k in range(KT):
                    nc.vector.tensor_copy(wv_bf[:, k * C:(k + 1) * C],
                                          v_stages.pop(k)[:])
                for p in (2, 3, 4):
                    stripe_dma(p, "q")
                    stripe_dma(p, "k")

            # ---- qkv + attention emission helpers ----
            def emit_qk_pair(p):
                # q: one accumulation over chunks, full-width evac (packed)
                q_ps = psum.tile([128, N], F32, tag="ps")
                for k in range(KT):
                    lhsT = wqk_bf[:, p * C + k * 128: p * C + (k + 1) * 128]
                    for off, w in halves(N):
                        nc.tensor.matmul(q_ps[:, off:off + w], lhsT,
                                         xt_bf[:, k * N + off: k * N + off + w],
                                         start=(k == 0), stop=(k == KT - 1))
                nc.vector.tensor_copy(qp_bf[:, p * N:(p + 1) * N], q_ps[:])
                # k: partition-aligned evac into the padded per-head blocks
                k_ps = psum.tile([128, N], F32, tag="ps")
                for k in range(KT):
                    lhsT = wqk_bf[:, (6 + p) * C + k * 128:
                                  (6 + p) * C + (k + 1) * 128]
                    for off, w in halves(N):
                        nc.tensor.matmul(k_ps[:, off:off + w], lhsT,
                                         xt_bf[:, k * N + off: k * N + off + w],
                                         start=(k == 0), stop=(k == KT - 1))
                h0, h1 = 2 * p, 2 * p + 1
                nc.gpsimd.memset(kt_pad[64:128, h0 * N:(h0 + 1) * N], 0.0)
                nc.gpsimd.memset(kt_pad[0:64, h1 * N:(h1 + 1) * N], 0.0)
                nc.vector.tensor_copy(kt_pad[0:64, h0 * N:(h0 + 1) * N],
                                      k_ps[0:64, :])
                nc.vector.tensor_copy(kt_pad[64:128, h1 * N:(h1 + 1) * N],
                                      k_ps[64:128, :])

            def emit_qk_half(p, which, hf):
                off = hf * 512
                base = (p if which == "q" else 6 + p) * C
                ps = psum.tile([128, 512], F32, tag="ps")
                for k in range(KT):
                    lhsT = wqk_bf[:, base + k * 128: base + (k + 1) * 128]
                    nc.tensor.matmul(ps[:, 0:512], lhsT,
                                     xt_bf[:, k * N + off: k * N + off + 512],
                                     start=(k == 0), stop=(k == KT - 1))
                if which == "q":
                    nc.vector.tensor_copy(qp_bf[:, p * N + off: p * N + off + 512],
                                          ps[:])
                else:
                    h0, h1 = 2 * p, 2 * p + 1
                    if hf == 0:
                        nc.gpsimd.memset(kt_pad[64:128, h0 * N:(h0 + 1) * N], 0.0)
                        nc.gpsimd.memset(kt_pad[0:64, h1 * N:(h1 + 1) * N], 0.0)
                    nc.vector.tensor_copy(
                        kt_pad[0:64, h0 * N + off: h0 * N + off + 512],
                        ps[0:64, :])
                    nc.vector.tensor_copy(
                        kt_pad[64:128, h1 * N + off: h1 * N + off + 512],
                        ps[64:128, :])

            e_tiles = {}

            def emit_scores_s(h, s):
                p = h // 2
                s_ps = psum.tile([128, N], F32, tag="ps")
                for off, w in halves(N):
                    nc.tensor.matmul(
                        s_ps[:, off:off + w],
                        kt_pad[:, h * N + s * 128: h * N + (s + 1) * 128],
                        qp_bf[:, p * N + off: p * N + off + w],
                        start=True, stop=True)
                e_t = expp.tile([128, N], BF16, tag="exp")
                nc.scalar.activation(e_t[:], s_ps[:], EXP_FN,
                                     bias=0.0, scale=SCALE)
                e_tiles.setdefault(h, []).append(e_t)


            def emit_v_tile(t):
                v_ps = psum.tile([128, C], F32, tag="ps")
                for k in range(KT):
                    lhsT = xt_bf[:, k * N + t * 128: k * N + (t + 1) * 128]
                    for off, w in halves(C):
                        nc.tensor.matmul(v_ps[:, off:off + w], lhsT,
                                         wv_bf[:, k * C + off: k * C + off + w],
                                         start=(k == 0), stop=(k == KT - 1))
                base = t * H * 65
                v_view = v65[:, base: base + H * 65].rearrange(
                    "p (h w) -> p h w", w=65)
                nc.vector.tensor_copy(
                    v_view[:, :, 0:64],
                    v_ps[:].rearrange("p (h d) -> p h d", d=64))
                nc.gpsimd.memset(v65_v[:, t, :, 64:65], 1.0)

            def emit_av_t(h, t):
                tiles = e_tiles[h]
                y_ps = psum.tile([128, 65], F32, tag="ys", bufs=2)
                for s in range(ST):
                    lhsT = tiles[s][:, t * 128:(t + 1) * 128]
                    rhs = v65[:, (s * H + h) * 65: (s * H + h + 1) * 65]
                    nc.tensor.matmul(y_ps[:, 0:65], lhsT, rhs,
                                     start=(s == 0), stop=(s == ST - 1))
                recip = recipp.tile([128, 1], F32, tag="recip", bufs=4)
                nc.vector.reciprocal(recip[:, 0:1], y_ps[:, 64:65])
                dst = y_nat[:, t * C + h * 64: t * C + (h + 1) * 64]
                nc.vector.tensor_scalar_mul(dst, y_ps[:, 0:64],
                                            recip[:, 0:1])
                if t == TT - 1:
                    e_tiles.pop(h)

            def emit_ytrans_t(i, t):
                tp = psum.tile([128, 128], BF16, tag="ps")
                nc.tensor.transpose(
                    tp[:], y_nat[:, t * C + i * 128: t * C + (i + 1) * 128],
                    ident[:])
                nc.vector.tensor_copy(
                    yt_bf[:, i * N + t * 128: i * N + (t + 1) * 128], tp[:])

            # ---- phase: qkv head-pair 0 + b broadcast ----
            _qs = nc.enter_named_scope("qkv", False)
            # b broadcast to 128 partitions via fp32 PE matmul (tiny, also
            # warms the PE pstate ramp before the dense qk stream)
            b_ps = psum.tile([128, C], F32, tag="ps")
            for off, w in halves(C):
                nc.tensor.matmul(b_ps[:, off:off + w], ones_f[0:1, 0:128],
                                 b_sb[0:1, off:off + w], start=True, stop=True)
            nc.scalar.copy(b_bcast[:], b_ps[:])
            emit_qk_pair(0)

            emit_qk_pair(1)
            stripe_cv(2)
            stripe_dma(5, "q")
            stripe_dma(5, "k")
            for i in range(TT):
                emit_scores_s(0, i)
                emit_v_tile(i)
            for i in range(TT):
                emit_scores_s(1, i)
                if i >= 1:
                    emit_av_t(0, i - 1)
            nc.leave_named_scope("qkv", _qs[0], False)

            # ---- phase: attention, software-pipelined across heads ----
            # scores(h) and AV(h-1) interleave per tile index so the AV
            # results arrive spread out (less DVE head-blocking) and the PE
            # stream stays dense (pstate ramp: any PE gap resets the clock
            # to mid for ~3us)
            _as = nc.enter_named_scope("attn", False)
            emit_av_t(0, TT - 1)
            for h in range(2, H):
                spread_pair = h // 2 + 1 if h in (2, 4, 6, 8) else None
                for i in range(TT):
                    emit_scores_s(h, i)
                    emit_av_t(h - 1, i)
                    if h >= 3 and h % 2 == 1:
                        emit_ytrans_t((h - 3) // 2, i)
                    if spread_pair is not None and i % 2 == 1:
                        which, hf = (("q", 0), ("q", 1), ("k", 0), ("k", 1))[i // 2]
                        emit_qk_half(spread_pair, which, hf)
                if h in (2, 4, 6):
                    # stripe converts + wp DMAs run two heads ahead of the
                    # pair matmuls so the PE never waits on them
                    stripe_cv(h // 2 + 2)
                    if h < 6:
                        wpchunk_dma(h - 2)
                        wpchunk_dma(h - 1)
                if h == 8:
                    wpchunk_dma(4)
                    wpchunk_dma(5)
                if h in (9, 10, 11):
                    k0 = 2 * (h - 9)
                    nc.vector.tensor_copy(wp_bf[:, k0 * C:(k0 + 1) * C],
                                          wp_stages.pop(k0)[:])
                    nc.vector.tensor_copy(wp_bf[:, (k0 + 1) * C:(k0 + 2) * C],
                                          wp_stages.pop(k0 + 1)[:])
            for i in range(TT):
                emit_av_t(H - 1, i)
                if i >= 1:
                    emit_ytrans_t(5, i - 1)
            emit_ytrans_t(5, TT - 1)
            nc.leave_named_scope("attn", _as[0], False)

            # ---- phase: out = yT^T @ W_proj + b ----
            _ps_ = nc.enter_named_scope("proj", False)
            for t in range(TT):
                z_ps = psum.tile([128, C], F32, tag="ps")
                for k in range(KT):
                    lhsT = yt_bf[:, k * N + t * 128: k * N + (t + 1) * 128]
                    for off, w in halves(C):
                        nc.tensor.matmul(z_ps[:, off:off + w], lhsT,
                                         wp_bf[:, k * C + off: k * C + off + w],
                                         start=(k == 0), stop=(k == KT - 1))
                z_sb = zp.tile([128, C], F32, tag="z")
                nc.vector.tensor_add(z_sb[:], z_ps[:], b_bcast[:])
                nc.scalar.dma_start(out_ext[t * 128:(t + 1) * 128, :], z_sb[:])
            nc.leave_named_scope("proj", _ps_[0], False)

    nc.finalize()
    return nc


_NC = None


def _get_nc():
    global _NC
    if _NC is None:
        _NC = build_nc()
    return _NC


def _run(x, W_qkv, W_proj, b_proj, trace=False):
    nc = _get_nc()
    W_qkv = np.ascontiguousarray(W_qkv, dtype=np.float32)
    W_proj = np.ascontiguousarray(W_proj, dtype=np.float32)
    b_proj = np.ascontiguousarray(b_proj, dtype=np.float32)
    in_maps = [
        {
            "x": np.ascontiguousarray(x[i], dtype=np.float32),
            "W_qkv": W_qkv,
            "W_proj": W_proj,
            "b_proj": b_proj,
        }
        for i in range(N_CORES)
    ]
    res = run_bass_kernel_spmd(nc, in_maps, core_ids=list(range(N_CORES)),
                               trace=trace)
    out = np.stack([res.results[i]["out"] for i in range(N_CORES)], axis=0)
    return out.astype(np.float32), res


def kernel(x, W_qkv, W_proj, b_proj):
    out, _ = _run(x, W_qkv, W_proj, b_proj, trace=False)
    return out
